# revision 24
# baseline (speedup 1.0000x reference)
"""3-layer GCN (Kipf GraphConvolution) on 8 Trainium2 NeuronCores.

Math per layer: h = relu(adj @ (h @ W) + b); final out = relu(h3 + x).

v4 architecture (row-shard / 1D node partition, fp8-resident adj):
  - adj is transposed on the host, quantized to fp8e4 (e4m3), and core c's
    shard adjT[:, c*NS:(c+1)*NS] is kept RESIDENT in SBUF: one initial load
    instead of streaming bf16 three times. Numerically validated: e4m3(adj)
    adds nothing over the bf16 error floor.
  - PADDED node ordering: the contraction axis uses a virtual ordering of
    8x1280 nodes (each rank's 1250 real nodes + 30 zero rows). Every
    contraction tile is then a full 128-row tile owned by exactly one rank,
    which makes all AllGather buffers plain contiguous slabs (no
    rearranging repacks, no short-tile edge cases). adj/s1 pad rows are
    zeroed on the host; the on-device support stage zeroes its pad rows
    once per layer so no NaN garbage can propagate through 0*x.
  - Support-AllGather: s_local = h_loc.T @ W is computed on the PRODUCING
    core (its h shard is local, ~2us of tiny matmuls BEFORE the
    collective), and the AllGather moves the SUPPORT in tile-major layout.
    The gathered buffer lands directly in agg-lhsT layout with one
    contiguous DMA per rank-block: the entire post-collective
    build/transpose/distribute phase of v2 is gone.
  - Layer-3's 320KB support AllGather is SPLIT into two pipelined meshes;
    the L3 aggregation is reordered to run the first half's tiles while
    the second mesh is still in flight.
  - Layers 1/2 aggregation (fout=20): 4-way PE column tiling, four
    j-tiles' stationaries at tile_position (0, 32g) run concurrently.
  - Layer 3 aggregation (fout=128): 4-way column tiling over feature
    quarters; each stationary streams all 3 chunks (LDWEIGHTS amortized).
  - One tiny warm-up AllGather pays the ncfw first-collective wake-up
    during the adj load; ncfw only begins mesh k when doorbell k+1 lands
    (or ~1us after mesh k-1 ends), so fewer queued warmups = less serial
    mesh time ahead of the real collectives.
"""

import math
import os

import numpy as np

# Experiment: default RDH collective path (no NEURON_RT_DBG_RDH_CC=0).

import concourse.bacc as bacc
import concourse.bass_utils as _bass_utils
import concourse.mybir as mybir
import concourse.tile as tile
from concourse.bass_utils import run_bass_kernel_spmd


# Problem geometry (hardcoded per the harness contract).
N = 10000
D_IN = 128
F1 = 20
F2 = 20
D_OUT = 128
NCORES = 8
NS = N // NCORES  # 1250 real nodes per core
NJ = math.ceil(NS / 128)  # 10 node sub-tiles per core (9 full + 98)
NSTAIL = NS - 128 * (NJ - 1)  # 98
NSP = NJ * 128  # 1280 padded nodes per core
NT2 = NCORES * NJ  # 80 padded contraction tiles
ICHUNK = 512
TILES_PER_SLAB = 10
NSLAB = NT2 // TILES_PER_SLAB  # 8 slabs of 10 tiles (1.6 MB each):
# fewer, fatter DMAs keep the 2 HWDGE rings at HBM line rate.

F32 = mybir.dt.float32
BF16 = mybir.dt.bfloat16
FP8 = mybir.dt.float8e4

# Filled by kernel() so a harness/test can inspect HW timing.
LAST_RESULTS = None


def _chunks(total, step):
    out = []
    i = 0
    while i < total:
        out.append((i, min(step, total - i)))
        i += step
    return out


def build_program(n=N, ncores=NCORES):
    ns = n // ncores
    nt = NT2
    chunks = _chunks(ns, ICHUNK)
    relu = mybir.ActivationFunctionType.Relu
    act_copy = mybir.ActivationFunctionType.Copy

    nc = bacc.Bacc("TRN2", target_bir_lowering=False, debug=False)

    adjq = nc.dram_tensor("adjq", [128, nt * ns], FP8, kind="ExternalInput")
    s1q = nc.dram_tensor("s1q", [128, nt * F1], BF16, kind="ExternalInput")
    W2 = nc.dram_tensor("W2", [128, F2], BF16, kind="ExternalInput")
    W3 = nc.dram_tensor("W3", [128, D_OUT], BF16, kind="ExternalInput")
    b1 = nc.dram_tensor("b1", [F1, 1], F32, kind="ExternalInput")
    b2 = nc.dram_tensor("b2", [F2, 1], F32, kind="ExternalInput")
    b3 = nc.dram_tensor("b3", [D_OUT, 1], F32, kind="ExternalInput")
    xTs = nc.dram_tensor("xTs", [D_IN, ns], F32, kind="ExternalInput")
    outT = nc.dram_tensor("outT", [D_OUT, ns], F32, kind="ExternalOutput")

    with tile.TileContext(nc, num_cores=ncores) as tc:
        with (
            tc.tile_pool(name="const", bufs=1) as const,
            tc.tile_pool(name="adj", bufs=1) as adjpool,
            tc.tile_pool(name="h", bufs=1) as hpool,
            tc.tile_pool(name="red", bufs=2) as redpool,
            tc.tile_pool(name="psy", bufs=1, space="PSUM") as psy_pool,
            tc.tile_pool(name="pss", bufs=1, space="PSUM") as pss_pool,
            tc.tile_pool(name="dram", bufs=1, space="DRAM") as dpool,
        ):
            w2_sb = const.tile([128, F2], BF16, tag="w2")
            w3_sb = const.tile([128, D_OUT], BF16, tag="w3")
            b1_sb = const.tile([F1, 1], F32, tag="b1")
            b2_sb = const.tile([F2, 1], F32, tag="b2")
            b3_sb = const.tile([D_OUT, 1], F32, tag="b3")
            xts_sb = const.tile([D_IN, ns], F32, tag="xts")
            s1_sb = const.tile([128, nt * F1], BF16, tag="s1")
            # One tiny warm-up AllGather, triggered at t~1-2us (gpsimd
            # carries nothing else up front). Its mesh is kicked by the
            # first real collective's doorbell; more warmups would only
            # queue more serial mesh work ahead of the real ones.
            warm_in = dpool.tile([1, 4], F32, tag="warmin")
            warm_out = dpool.tile([ncores, 4], F32, tag="warmout",
                                  addr_space="Shared")
            nc.sync.dma_start(out=warm_in[:, :], in_=xTs[0:1, 0:4])
            nc.gpsimd.collective_compute(
                "AllGather",
                mybir.AluOpType.bypass,
                replica_groups=[list(range(ncores))],
                ins=[warm_in.opt()],
                outs=[warm_out.opt()],
            )

            # Const loads on the HWDGE rings (NOT gpsimd: SWDGE descriptor
            # generation for these was delaying the warm-up trigger ~18us).
            nc.sync.dma_start(out=s1_sb[:, :], in_=s1q[:, :])
            for sb, dr in (
                (w2_sb, W2),
                (w3_sb, W3),
                (b1_sb, b1),
                (b2_sb, b2),
                (b3_sb, b3),
            ):
                nc.scalar.dma_start(out=sb[:, :], in_=dr[:, :])

            # ---- resident adj: 16 slab loads alternating the 2 HWDGE rings
            slabs = []
            for s in range(NSLAB):
                at = adjpool.tile([128, TILES_PER_SLAB * ns], FP8,
                                  tag=f"slab{s}")
                eng = nc.sync if s % 2 == 0 else nc.scalar
                c0 = s * TILES_PER_SLAB * ns
                eng.dma_start(
                    out=at[:, :], in_=adjq[:, c0 : c0 + TILES_PER_SLAB * ns]
                )
                slabs.append(at)

            def adj_slice(t, i0, ilen):
                s, r = divmod(t, TILES_PER_SLAB)
                return slabs[s][:, r * ns + i0 : r * ns + i0 + ilen]

            # ---- col-tiled aggregation for fout<=32 (layers 1 and 2).
            def agg_tiled(s_tiles, fout, lname, chunk_major=False):
                lastt = [max(t for t in range(nt) if t % 4 == g)
                         for g in range(4)]
                psy = [
                    psy_pool.tile([128, ICHUNK], F32, tag=f"psy{ic}",
                                  name=f"psy_{lname}_{ic}")
                    for ic in range(len(chunks))
                ]
                if chunk_major:
                    order = [(ic, t) for ic in range(len(chunks))
                             for t in range(nt)]
                else:
                    order = [(ic, b0 + g) for b0 in range(0, nt, 4)
                             for ic in range(len(chunks)) for g in range(4)
                             if b0 + g < nt]
                for ic, t in order:
                    i0, ilen = chunks[ic]
                    g = t % 4
                    nc.tensor.matmul(
                        psy[ic][32 * g : 32 * g + fout, :ilen],
                        lhsT=s_tiles[t][:, :fout],
                        rhs=adj_slice(t, i0, ilen),
                        start=(t < 4),
                        stop=(t == lastt[g]),
                        tile_position=(0, 32 * g),
                    )
                return psy

            def reduce_relu(psy, fout, dst, b_sb):
                """dst[:, i0:i0+ilen] = relu(sum of 4 col-group slices + b).

                Each tensor op may read at most ONE PSUM operand: ACT
                evacuates group 0 while DVE chains the other three."""
                for ic, (i0, ilen) in enumerate(chunks):
                    a = redpool.tile([F1, ICHUNK], F32, tag="ra",
                                     name=f"ra{ic}")
                    b = redpool.tile([F1, ICHUNK], F32, tag="rb",
                                     name=f"rb{ic}")
                    nc.scalar.activation(a[:fout, :ilen],
                                         psy[ic][0:fout, :ilen], act_copy)
                    nc.vector.tensor_add(b[:fout, :ilen], a[:fout, :ilen],
                                         psy[ic][32 : 32 + fout, :ilen])
                    nc.vector.tensor_add(a[:fout, :ilen], b[:fout, :ilen],
                                         psy[ic][64 : 64 + fout, :ilen])
                    nc.vector.tensor_add(b[:fout, :ilen], a[:fout, :ilen],
                                         psy[ic][96 : 96 + fout, :ilen])
                    nc.scalar.activation(dst[:fout, i0 : i0 + ilen],
                                         b[:fout, :ilen], relu,
                                         bias=b_sb[:fout, :])

            def support_allgather(h_loc, w_sb, fin, fout, layer, parts):
                """s_local = h_loc.T @ W on the PRODUCING core, AllGather
                the SUPPORT in tile-major layout (all buffers contiguous
                thanks to the 1280-padded node ordering). `parts` splits the
                local j-range into multiple pipelined collectives; returns
                (s_tiles, order) where order lists tile indices in
                arrival order."""
                s_stage = hpool.tile([128, NJ * fout], BF16,
                                     tag=f"sstage{layer}")
                # Zero the 30 pad rows of the last j-block once: pad adj
                # rows are zero but 0*NaN = NaN, so the stage must hold
                # finite values there. (Partition base must be 32-aligned;
                # rows 96..97 are rewritten by the j=9 copy below.)
                nc.vector.memset(s_stage[96:, (NJ - 1) * fout :], 0.0)
                for j in range(NJ):
                    mj = min(128, ns - j * 128)
                    ps = pss_pool.tile([128, 512], F32, tag=f"psl{j % 2}",
                                       name=f"psl_{layer}_{j}")
                    nc.tensor.matmul(
                        ps[:mj, :fout],
                        lhsT=h_loc[:fin, j * 128 : j * 128 + mj],
                        rhs=w_sb[:fin, :fout],
                        start=True,
                        stop=True,
                    )
                    if j % 2 == 0:
                        nc.vector.tensor_copy(
                            s_stage[:mj, j * fout : (j + 1) * fout],
                            ps[:mj, :fout],
                        )
                    else:
                        nc.scalar.activation(
                            s_stage[:mj, j * fout : (j + 1) * fout],
                            ps[:mj, :fout],
                            act_copy,
                        )
                s_full = hpool.tile([128, nt * fout], BF16,
                                    tag=f"sfull{layer}")
                order = []
                # All bounce-in DMAs and doorbells FIRST: a landing DMA
                # waits on its mesh, and HWDGE rings are FIFO per engine,
                # so a later part's cc_in must not queue behind an earlier
                # part's landing.
                ccs = []
                for pi, (j0, j1) in enumerate(parts):
                    w = j1 - j0
                    cc_in = dpool.tile([128, w * fout], BF16,
                                       tag=f"ccin{layer}_{pi}")
                    cc_out = dpool.tile([128 * ncores, w * fout], BF16,
                                        tag=f"ccout{layer}_{pi}",
                                        addr_space="Shared")
                    nc.sync.dma_start(
                        out=cc_in[:, :],
                        in_=s_stage[:, j0 * fout : j1 * fout],
                    )
                    ccs.append((j0, j1, cc_in, cc_out))
                for j0, j1, cc_in, cc_out in ccs:
                    nc.gpsimd.collective_compute(
                        "AllGather",
                        mybir.AluOpType.bypass,
                        replica_groups=[list(range(ncores))],
                        ins=[cc_in.opt()],
                        outs=[cc_out.opt()],
                    )
                for j0, j1, cc_in, cc_out in ccs:
                    # Land rank-blocks into tile-major s_full: tile index of
                    # (rank r, local j) is r*NJ + j. Per-partition runs are
                    # w*fout contiguous elements -> clean fat descriptors.
                    for r0, r1, eng in ((0, 4, nc.sync), (4, 8, nc.scalar)):
                        eng.dma_start(
                            out=s_full[:, :].rearrange(
                                "p (r x) -> p r x", r=ncores
                            )[:, r0:r1, j0 * fout : j1 * fout],
                            in_=cc_out[:, :].rearrange(
                                "(r p) x -> p r x", p=128
                            )[:, r0:r1, :],
                        )
                    order.extend(r * NJ + j for r in range(ncores)
                                 for j in range(j0, j1))
                tiles = [s_full[:, t * fout : (t + 1) * fout]
                         for t in range(nt)]
                return tiles, order

            # ---- Layer 1: s1 from host (padded tile order) ----
            s1_tiles = [s1_sb[:, t * F1 : (t + 1) * F1] for t in range(nt)]
            psy1 = agg_tiled(s1_tiles, F1, "l1")
            h1_loc = hpool.tile([F1, ns], BF16, tag="hloc1")
            reduce_relu(psy1, F1, h1_loc, b1_sb)
            s2_tiles, l2_order = support_allgather(
                h1_loc, w2_sb, F1, F2, 2, parts=[(0, NJ // 2), (NJ // 2, NJ)]
            )

            # ---- Layer 2 (chunk-major banks; tiles in AllGather-arrival
            # order so the first half aggregates while mesh b is in
            # flight; col group = position in arrival order mod 4) ----
            psy2 = [
                psy_pool.tile([128, ICHUNK], F32, tag=f"psy{ic}",
                              name=f"psy_l2_{ic}")
                for ic in range(len(chunks))
            ]
            for ic, (i0, ilen) in enumerate(chunks):
                for pos, t in enumerate(l2_order):
                    g = pos % 4
                    nc.tensor.matmul(
                        psy2[ic][32 * g : 32 * g + F2, :ilen],
                        lhsT=s2_tiles[t][:, :F2],
                        rhs=adj_slice(t, i0, ilen),
                        start=(pos < 4),
                        stop=(pos >= nt - 4),
                        tile_position=(0, 32 * g),
                    )
            h2_loc = hpool.tile([F2, ns], BF16, tag="hloc2")
            reduce_relu(psy2, F2, h2_loc, b2_sb)
            s3_tiles, l3_order = support_allgather(
                h2_loc, w3_sb, F2, D_OUT, 3, parts=[(0, NJ // 2), (NJ // 2, NJ)]
            )

            # ---- Layer 3 (fout=128, 4-way PE column tiling over feature
            # quarters; tiles processed in AllGather-arrival order so the
            # first half aggregates while the second mesh is in flight) ----
            psy3 = [
                psy_pool.tile([128, ICHUNK], F32, tag=f"psy{ic}",
                              name=f"psy3_{ic}")
                for ic in range(len(chunks))
            ]
            for idx, t in enumerate(l3_order):
                for ic, (i0, ilen) in enumerate(chunks):
                    for g in range(4):
                        nc.tensor.matmul(
                            psy3[ic][32 * g : 32 * g + 32, :ilen],
                            lhsT=s3_tiles[t][:, 32 * g : 32 * g + 32],
                            rhs=adj_slice(t, i0, ilen),
                            start=(idx == 0),
                            stop=(idx == nt - 1),
                            tile_position=(0, 32 * g),
                        )

            # xts is only needed by the epilogue below; load it late so it
            # doesn't compete with adj/s1 for HBM bandwidth at kernel start.
            nc.scalar.dma_start(out=xts_sb[:, :], in_=xTs[:, :])

            # ---- out = relu(h3 + x); each chunk's store DMA is issued as
            # soon as its relu lands instead of one big trailing DMA ----
            o_sb = hpool.tile([D_OUT, ns], F32, tag="ostage")
            for ic, (i0, ilen) in enumerate(chunks):
                a = redpool.tile([D_OUT, ICHUNK], F32, tag="fa", name=f"fa{ic}")
                nc.vector.tensor_add(a[:, :ilen], xts_sb[:, i0 : i0 + ilen],
                                     psy3[ic][:, :ilen])
                nc.scalar.activation(o_sb[:, i0 : i0 + ilen], a[:, :ilen], relu,
                                     bias=b3_sb[:, :])
                nc.sync.dma_start(out=outT[:, i0 : i0 + ilen],
                                  in_=o_sb[:, i0 : i0 + ilen])

    nc.compile()
    return nc


def _ensure_ntff_hook():
    """Register the axon NTFF profile hook if the image's antenv lacks it."""
    import contextlib
    import ctypes
    import sys
    import types

    try:
        from antenv.axon_hooks import get_axon_ntff_profile_hook  # noqa: F401

        return
    except ImportError:
        pass

    so_path = "/opt/axon/libaxon_pjrt.so"
    lib = ctypes.CDLL(so_path)
    if not hasattr(lib, "axon_start_nrt_profile"):
        return
    lib.axon_start_nrt_profile.argtypes = [
        ctypes.POINTER(ctypes.c_int64),
        ctypes.c_size_t,
    ]
    lib.axon_start_nrt_profile.restype = ctypes.c_int64
    lib.axon_stop_nrt_profile.argtypes = [ctypes.c_char_p]
    lib.axon_stop_nrt_profile.restype = ctypes.c_int64

    @contextlib.contextmanager
    def _hook(output_dir, device_ids):
        import jax

        jax.devices()
        if device_ids:
            ids = (ctypes.c_int64 * len(device_ids))(*device_ids)
            rc = lib.axon_start_nrt_profile(ids, len(device_ids))
        else:
            rc = lib.axon_start_nrt_profile(None, 0)
        if rc != 0:
            raise RuntimeError(f"axon_start_nrt_profile rc={rc}")
        try:
            yield
        finally:
            n = lib.axon_stop_nrt_profile(str(output_dir).encode())
            print(f"ntff profile: {n} file(s) written to {output_dir}")

    mod = types.ModuleType("antenv.axon_hooks")
    _state = {"hook": _hook}
    mod.get_axon_ntff_profile_hook = lambda: _state["hook"]
    mod.set_axon_ntff_profile_hook = lambda h: _state.update(hook=h)
    sys.modules["antenv.axon_hooks"] = mod
    import antenv

    antenv.axon_hooks = mod


_PROGRAM = None


def _get_program():
    global _PROGRAM
    if _PROGRAM is None:
        _PROGRAM = build_program()
    return _PROGRAM


def _replicate4(w):
    """Stack w [f, g] at partition offsets 0/32/64/96 of a [128, g] array."""
    f, g = w.shape
    out = np.zeros((128, g), np.float32)
    for r in range(4):
        out[32 * r : 32 * r + f] = w
    return out


def _pad_rows(a, dtype):
    """[N, k] -> [NCORES*NSP, k] with each rank's 1250 rows at 1280-stride
    (pad rows zero)."""
    out = np.zeros((NCORES * NSP, a.shape[1]), dtype)
    for r in range(NCORES):
        out[NSP * r : NSP * r + NS] = a[NS * r : NS * r + NS]
    return out


def kernel(**inputs):
    global LAST_RESULTS
    np_fp8 = mybir.dt.np(FP8)
    np_bf16 = mybir.dt.np(BF16)

    x = np.asarray(inputs["x"], dtype=np.float32)
    adj = np.asarray(inputs["adj"], dtype=np.float32)
    W1 = np.asarray(inputs["W1"], np.float32)
    b1 = np.asarray(inputs["b1"], np.float32)

    adjT_q = np.ascontiguousarray(adj.T).astype(np_fp8)
    xT = np.ascontiguousarray(x.T)

    # Host-side s1 = x @ W1 in the padded tile layout.
    s1_pad = _pad_rows(x @ W1, np.float32)
    s1_tiled = np.ascontiguousarray(
        s1_pad.reshape(NT2, 128, F1).transpose(1, 0, 2).reshape(128, NT2 * F1)
    ).astype(np_bf16)

    base = {
        "s1q": s1_tiled,
        "W2": _replicate4(np.asarray(inputs["W2"], np.float32)).astype(np_bf16),
        "W3": _replicate4(np.asarray(inputs["W3"], np.float32)).astype(np_bf16),
        "b1": b1.reshape(F1, 1),
        "b2": np.asarray(inputs["b2"], np.float32).reshape(F2, 1),
        "b3": np.asarray(inputs["b3"], np.float32).reshape(D_OUT, 1),
    }
    in_maps = []
    for c in range(NCORES):
        sl = slice(c * NS, (c + 1) * NS)
        pad = _pad_rows(adjT_q[:, sl], np_fp8)
        adj_tiled = np.ascontiguousarray(
            pad.reshape(NT2, 128, NS).transpose(1, 0, 2).reshape(128, NT2 * NS)
        )
        in_maps.append(
            dict(
                base,
                adjq=adj_tiled,
                xTs=np.ascontiguousarray(xT[:, sl]),
            )
        )

    nc = _get_program()
    trace = bool(int(os.environ.get("GCN_TRACE", "0")))
    extra = {}
    if trace:
        _ensure_ntff_hook()
        if os.environ.get("GCN_TRACE_DIR"):
            os.makedirs(os.environ["GCN_TRACE_DIR"], exist_ok=True)
            extra["tmpdir"] = os.environ["GCN_TRACE_DIR"]
    LAST_RESULTS = run_bass_kernel_spmd(
        nc, in_maps, list(range(NCORES)), trace=trace, **extra
    )
    out = np.concatenate(
        [np.asarray(LAST_RESULTS.results[c]["outT"]).T for c in range(NCORES)],
        axis=0,
    )
    return np.ascontiguousarray(out.astype(np.float32))


# revision 25
# speedup vs baseline: 1.0521x; 1.0521x over previous
"""3-layer GCN (Kipf GraphConvolution) on 8 Trainium2 NeuronCores.

Math per layer: h = relu(adj @ (h @ W) + b); final out = relu(h3 + x).

v4 architecture (row-shard / 1D node partition, fp8-resident adj):
  - adj is transposed on the host, quantized to fp8e4 (e4m3), and core c's
    shard adjT[:, c*NS:(c+1)*NS] is kept RESIDENT in SBUF: one initial load
    instead of streaming bf16 three times. Numerically validated: e4m3(adj)
    adds nothing over the bf16 error floor.
  - PADDED node ordering: the contraction axis uses a virtual ordering of
    8x1280 nodes (each rank's 1250 real nodes + 30 zero rows). Every
    contraction tile is then a full 128-row tile owned by exactly one rank,
    which makes all AllGather buffers plain contiguous slabs (no
    rearranging repacks, no short-tile edge cases). adj/s1 pad rows are
    zeroed on the host; the on-device support stage zeroes its pad rows
    once per layer so no NaN garbage can propagate through 0*x.
  - Support-AllGather: s_local = h_loc.T @ W is computed on the PRODUCING
    core (its h shard is local, ~2us of tiny matmuls BEFORE the
    collective), and the AllGather moves the SUPPORT in tile-major layout.
    The gathered buffer lands directly in agg-lhsT layout with one
    contiguous DMA per rank-block: the entire post-collective
    build/transpose/distribute phase of v2 is gone.
  - Layer-3's 320KB support AllGather is SPLIT into two pipelined meshes;
    the L3 aggregation is reordered to run the first half's tiles while
    the second mesh is still in flight.
  - Layers 1/2 aggregation (fout=20): 4-way PE column tiling, four
    j-tiles' stationaries at tile_position (0, 32g) run concurrently.
  - Layer 3 aggregation (fout=128): 4-way column tiling over feature
    quarters; each stationary streams all 3 chunks (LDWEIGHTS amortized).
  - One tiny warm-up AllGather pays the ncfw first-collective wake-up
    during the adj load; ncfw only begins mesh k when doorbell k+1 lands
    (or ~1us after mesh k-1 ends), so fewer queued warmups = less serial
    mesh time ahead of the real collectives.
"""

import math
import os

import numpy as np

# Experiment: default RDH collective path (no NEURON_RT_DBG_RDH_CC=0).

import concourse.bacc as bacc
import concourse.bass_utils as _bass_utils
import concourse.mybir as mybir
import concourse.tile as tile
from concourse.bass_utils import run_bass_kernel_spmd


# Problem geometry (hardcoded per the harness contract).
N = 10000
D_IN = 128
F1 = 20
F2 = 20
D_OUT = 128
NCORES = 8
NS = N // NCORES  # 1250 real nodes per core
NJ = math.ceil(NS / 128)  # 10 node sub-tiles per core (9 full + 98)
NSTAIL = NS - 128 * (NJ - 1)  # 98
NSP = NJ * 128  # 1280 padded nodes per core
NT2 = NCORES * NJ  # 80 padded contraction tiles
ICHUNK = 512
TILES_PER_SLAB = 10
NSLAB = NT2 // TILES_PER_SLAB  # 8 slabs of 10 tiles (1.6 MB each):
# fewer, fatter DMAs keep the 2 HWDGE rings at HBM line rate.

F32 = mybir.dt.float32
BF16 = mybir.dt.bfloat16
FP8 = mybir.dt.float8e4

# Filled by kernel() so a harness/test can inspect HW timing.
LAST_RESULTS = None


def _chunks(total, step):
    out = []
    i = 0
    while i < total:
        out.append((i, min(step, total - i)))
        i += step
    return out


def build_program(n=N, ncores=NCORES):
    ns = n // ncores
    nt = NT2
    chunks = _chunks(ns, ICHUNK)
    relu = mybir.ActivationFunctionType.Relu
    act_copy = mybir.ActivationFunctionType.Copy

    nc = bacc.Bacc("TRN2", target_bir_lowering=False, debug=False)

    adjq = nc.dram_tensor("adjq", [128, nt * ns], FP8, kind="ExternalInput")
    s1q = nc.dram_tensor("s1q", [128, nt * F1], BF16, kind="ExternalInput")
    W2 = nc.dram_tensor("W2", [128, F2], BF16, kind="ExternalInput")
    W3 = nc.dram_tensor("W3", [128, D_OUT], BF16, kind="ExternalInput")
    b1 = nc.dram_tensor("b1", [F1, 1], F32, kind="ExternalInput")
    b2 = nc.dram_tensor("b2", [F2, 1], F32, kind="ExternalInput")
    b3 = nc.dram_tensor("b3", [D_OUT, 1], F32, kind="ExternalInput")
    xTs = nc.dram_tensor("xTs", [D_IN, ns], F32, kind="ExternalInput")
    outT = nc.dram_tensor("outT", [D_OUT, ns], F32, kind="ExternalOutput")

    with tile.TileContext(nc, num_cores=ncores) as tc:
        with (
            tc.tile_pool(name="const", bufs=1) as const,
            tc.tile_pool(name="adj", bufs=1) as adjpool,
            tc.tile_pool(name="h", bufs=1) as hpool,
            tc.tile_pool(name="red", bufs=2) as redpool,
            tc.tile_pool(name="psy", bufs=1, space="PSUM") as psy_pool,
            tc.tile_pool(name="pss", bufs=1, space="PSUM") as pss_pool,
            tc.tile_pool(name="dram", bufs=1, space="DRAM") as dpool,
        ):
            w2_sb = const.tile([128, F2], BF16, tag="w2")
            w3_sb = const.tile([128, D_OUT], BF16, tag="w3")
            b1_sb = const.tile([F1, 1], F32, tag="b1")
            b2_sb = const.tile([F2, 1], F32, tag="b2")
            b3_sb = const.tile([D_OUT, 1], F32, tag="b3")
            xts_sb = const.tile([D_IN, ns], F32, tag="xts")
            s1_sb = const.tile([128, nt * F1], BF16, tag="s1")
            # One tiny warm-up AllGather, triggered at t~1-2us (gpsimd
            # carries nothing else up front). Its mesh is kicked by the
            # first real collective's doorbell; more warmups would only
            # queue more serial mesh work ahead of the real ones.
            warm_in = dpool.tile([1, 4], F32, tag="warmin")
            warm_out = dpool.tile([ncores, 4], F32, tag="warmout",
                                  addr_space="Shared")
            nc.sync.dma_start(out=warm_in[:, :], in_=xTs[0:1, 0:4])
            nc.gpsimd.collective_compute(
                "AllGather",
                mybir.AluOpType.bypass,
                replica_groups=[list(range(ncores))],
                ins=[warm_in.opt()],
                outs=[warm_out.opt()],
            )

            # Const loads on the HWDGE rings (NOT gpsimd: SWDGE descriptor
            # generation for these was delaying the warm-up trigger ~18us).
            nc.sync.dma_start(out=s1_sb[:, :], in_=s1q[:, :])
            for sb, dr in (
                (w2_sb, W2),
                (w3_sb, W3),
                (b1_sb, b1),
                (b2_sb, b2),
                (b3_sb, b3),
            ):
                nc.scalar.dma_start(out=sb[:, :], in_=dr[:, :])

            # ---- resident adj: 16 slab loads alternating the 2 HWDGE rings
            slabs = []
            for s in range(NSLAB):
                at = adjpool.tile([128, TILES_PER_SLAB * ns], FP8,
                                  tag=f"slab{s}")
                eng = nc.sync if s % 2 == 0 else nc.scalar
                c0 = s * TILES_PER_SLAB * ns
                eng.dma_start(
                    out=at[:, :], in_=adjq[:, c0 : c0 + TILES_PER_SLAB * ns]
                )
                slabs.append(at)

            def adj_slice(t, i0, ilen):
                s, r = divmod(t, TILES_PER_SLAB)
                return slabs[s][:, r * ns + i0 : r * ns + i0 + ilen]

            # ---- col-tiled aggregation for fout<=32 (layers 1 and 2).
            def agg_tiled(s_tiles, fout, lname, chunk_major=False):
                lastt = [max(t for t in range(nt) if t % 4 == g)
                         for g in range(4)]
                psy = [
                    psy_pool.tile([128, ICHUNK], F32, tag=f"psy{ic}",
                                  name=f"psy_{lname}_{ic}")
                    for ic in range(len(chunks))
                ]
                if chunk_major:
                    order = [(ic, t) for ic in range(len(chunks))
                             for t in range(nt)]
                else:
                    order = [(ic, b0 + g) for b0 in range(0, nt, 4)
                             for ic in range(len(chunks)) for g in range(4)
                             if b0 + g < nt]
                for ic, t in order:
                    i0, ilen = chunks[ic]
                    g = t % 4
                    nc.tensor.matmul(
                        psy[ic][32 * g : 32 * g + fout, :ilen],
                        lhsT=s_tiles[t][:, :fout],
                        rhs=adj_slice(t, i0, ilen),
                        start=(t < 4),
                        stop=(t == lastt[g]),
                        tile_position=(0, 32 * g),
                    )
                return psy

            def reduce_relu(psy, fout, dst, b_sb):
                """dst[:, i0:i0+ilen] = relu(sum of 4 col-group slices + b).

                Each tensor op may read at most ONE PSUM operand: ACT
                evacuates group 0 while DVE chains the other three."""
                for ic, (i0, ilen) in enumerate(chunks):
                    a = redpool.tile([F1, ICHUNK], F32, tag="ra",
                                     name=f"ra{ic}")
                    b = redpool.tile([F1, ICHUNK], F32, tag="rb",
                                     name=f"rb{ic}")
                    nc.scalar.activation(a[:fout, :ilen],
                                         psy[ic][0:fout, :ilen], act_copy)
                    nc.vector.tensor_add(b[:fout, :ilen], a[:fout, :ilen],
                                         psy[ic][32 : 32 + fout, :ilen])
                    nc.vector.tensor_add(a[:fout, :ilen], b[:fout, :ilen],
                                         psy[ic][64 : 64 + fout, :ilen])
                    nc.vector.tensor_add(b[:fout, :ilen], a[:fout, :ilen],
                                         psy[ic][96 : 96 + fout, :ilen])
                    nc.scalar.activation(dst[:fout, i0 : i0 + ilen],
                                         b[:fout, :ilen], relu,
                                         bias=b_sb[:fout, :])

            def support_allgather(h_loc, w_sb, fin, fout, layer, parts):
                """s_local = h_loc.T @ W on the PRODUCING core, AllGather
                the SUPPORT in tile-major layout (all buffers contiguous
                thanks to the 1280-padded node ordering). `parts` splits the
                local j-range into multiple pipelined collectives; returns
                (s_tiles, order) where order lists tile indices in
                arrival order."""
                s_stage = hpool.tile([128, NJ * fout], BF16,
                                     tag=f"sstage{layer}")
                # Zero the 30 pad rows of the last j-block once: pad adj
                # rows are zero but 0*NaN = NaN, so the stage must hold
                # finite values there. (Partition base must be 32-aligned;
                # rows 96..97 are rewritten by the j=9 copy below.)
                nc.vector.memset(s_stage[96:, (NJ - 1) * fout :], 0.0)
                for j in range(NJ):
                    mj = min(128, ns - j * 128)
                    ps = pss_pool.tile([128, 512], F32, tag=f"psl{j % 2}",
                                       name=f"psl_{layer}_{j}")
                    nc.tensor.matmul(
                        ps[:mj, :fout],
                        lhsT=h_loc[:fin, j * 128 : j * 128 + mj],
                        rhs=w_sb[:fin, :fout],
                        start=True,
                        stop=True,
                    )
                    if j % 2 == 0:
                        nc.vector.tensor_copy(
                            s_stage[:mj, j * fout : (j + 1) * fout],
                            ps[:mj, :fout],
                        )
                    else:
                        nc.scalar.activation(
                            s_stage[:mj, j * fout : (j + 1) * fout],
                            ps[:mj, :fout],
                            act_copy,
                        )
                s_full = hpool.tile([128, nt * fout], BF16,
                                    tag=f"sfull{layer}")
                order = []
                # All bounce-in DMAs and doorbells FIRST: a landing DMA
                # waits on its mesh, and HWDGE rings are FIFO per engine,
                # so a later part's cc_in must not queue behind an earlier
                # part's landing.
                ccs = []
                for pi, (j0, j1) in enumerate(parts):
                    w = j1 - j0
                    cc_in = dpool.tile([128, w * fout], BF16,
                                       tag=f"ccin{layer}_{pi}")
                    cc_out = dpool.tile([128 * ncores, w * fout], BF16,
                                        tag=f"ccout{layer}_{pi}",
                                        addr_space="Shared")
                    nc.sync.dma_start(
                        out=cc_in[:, :],
                        in_=s_stage[:, j0 * fout : j1 * fout],
                    )
                    ccs.append((j0, j1, cc_in, cc_out))
                for j0, j1, cc_in, cc_out in ccs:
                    nc.gpsimd.collective_compute(
                        "AllGather",
                        mybir.AluOpType.bypass,
                        replica_groups=[list(range(ncores))],
                        ins=[cc_in.opt()],
                        outs=[cc_out.opt()],
                    )
                for j0, j1, cc_in, cc_out in ccs:
                    # Land rank-blocks into tile-major s_full: tile index of
                    # (rank r, local j) is r*NJ + j. Per-partition runs are
                    # w*fout contiguous elements -> clean fat descriptors.
                    for r0, r1, eng in ((0, 4, nc.sync), (4, 8, nc.scalar)):
                        eng.dma_start(
                            out=s_full[:, :].rearrange(
                                "p (r x) -> p r x", r=ncores
                            )[:, r0:r1, j0 * fout : j1 * fout],
                            in_=cc_out[:, :].rearrange(
                                "(r p) x -> p r x", p=128
                            )[:, r0:r1, :],
                        )
                    order.extend(r * NJ + j for r in range(ncores)
                                 for j in range(j0, j1))
                tiles = [s_full[:, t * fout : (t + 1) * fout]
                         for t in range(nt)]
                return tiles, order

            # ---- Layer 1: s1 from host (padded tile order) ----
            s1_tiles = [s1_sb[:, t * F1 : (t + 1) * F1] for t in range(nt)]
            psy1 = agg_tiled(s1_tiles, F1, "l1")
            h1_loc = hpool.tile([F1, ns], BF16, tag="hloc1")
            reduce_relu(psy1, F1, h1_loc, b1_sb)
            s2_tiles, _ = support_allgather(h1_loc, w2_sb, F1, F2, 2,
                                            parts=[(0, NJ)])

            # ---- Layer 2 ----
            psy2 = agg_tiled(s2_tiles, F2, "l2", chunk_major=True)
            h2_loc = hpool.tile([F2, ns], BF16, tag="hloc2")
            reduce_relu(psy2, F2, h2_loc, b2_sb)
            s3_tiles, l3_order = support_allgather(
                h2_loc, w3_sb, F2, D_OUT, 3, parts=[(0, NJ // 2), (NJ // 2, NJ)]
            )

            # ---- Layer 3 (fout=128, 4-way PE column tiling over feature
            # quarters; tiles processed in AllGather-arrival order so the
            # first half aggregates while the second mesh is in flight) ----
            psy3 = [
                psy_pool.tile([128, ICHUNK], F32, tag=f"psy{ic}",
                              name=f"psy3_{ic}")
                for ic in range(len(chunks))
            ]
            for idx, t in enumerate(l3_order):
                for ic, (i0, ilen) in enumerate(chunks):
                    for g in range(4):
                        nc.tensor.matmul(
                            psy3[ic][32 * g : 32 * g + 32, :ilen],
                            lhsT=s3_tiles[t][:, 32 * g : 32 * g + 32],
                            rhs=adj_slice(t, i0, ilen),
                            start=(idx == 0),
                            stop=(idx == nt - 1),
                            tile_position=(0, 32 * g),
                        )

            # xts is only needed by the epilogue below; load it late so it
            # doesn't compete with adj/s1 for HBM bandwidth at kernel start.
            nc.scalar.dma_start(out=xts_sb[:, :], in_=xTs[:, :])

            # ---- out = relu(h3 + x); each chunk's store DMA is issued as
            # soon as its relu lands instead of one big trailing DMA ----
            o_sb = hpool.tile([D_OUT, ns], F32, tag="ostage")
            for ic, (i0, ilen) in enumerate(chunks):
                a = redpool.tile([D_OUT, ICHUNK], F32, tag="fa", name=f"fa{ic}")
                nc.vector.tensor_add(a[:, :ilen], xts_sb[:, i0 : i0 + ilen],
                                     psy3[ic][:, :ilen])
                nc.scalar.activation(o_sb[:, i0 : i0 + ilen], a[:, :ilen], relu,
                                     bias=b3_sb[:, :])
                nc.sync.dma_start(out=outT[:, i0 : i0 + ilen],
                                  in_=o_sb[:, i0 : i0 + ilen])

    nc.compile()
    return nc


def _ensure_ntff_hook():
    """Register the axon NTFF profile hook if the image's antenv lacks it."""
    import contextlib
    import ctypes
    import sys
    import types

    try:
        from antenv.axon_hooks import get_axon_ntff_profile_hook  # noqa: F401

        return
    except ImportError:
        pass

    so_path = "/opt/axon/libaxon_pjrt.so"
    lib = ctypes.CDLL(so_path)
    if not hasattr(lib, "axon_start_nrt_profile"):
        return
    lib.axon_start_nrt_profile.argtypes = [
        ctypes.POINTER(ctypes.c_int64),
        ctypes.c_size_t,
    ]
    lib.axon_start_nrt_profile.restype = ctypes.c_int64
    lib.axon_stop_nrt_profile.argtypes = [ctypes.c_char_p]
    lib.axon_stop_nrt_profile.restype = ctypes.c_int64

    @contextlib.contextmanager
    def _hook(output_dir, device_ids):
        import jax

        jax.devices()
        if device_ids:
            ids = (ctypes.c_int64 * len(device_ids))(*device_ids)
            rc = lib.axon_start_nrt_profile(ids, len(device_ids))
        else:
            rc = lib.axon_start_nrt_profile(None, 0)
        if rc != 0:
            raise RuntimeError(f"axon_start_nrt_profile rc={rc}")
        try:
            yield
        finally:
            n = lib.axon_stop_nrt_profile(str(output_dir).encode())
            print(f"ntff profile: {n} file(s) written to {output_dir}")

    mod = types.ModuleType("antenv.axon_hooks")
    _state = {"hook": _hook}
    mod.get_axon_ntff_profile_hook = lambda: _state["hook"]
    mod.set_axon_ntff_profile_hook = lambda h: _state.update(hook=h)
    sys.modules["antenv.axon_hooks"] = mod
    import antenv

    antenv.axon_hooks = mod


_PROGRAM = None


def _get_program():
    global _PROGRAM
    if _PROGRAM is None:
        _PROGRAM = build_program()
    return _PROGRAM


def _replicate4(w):
    """Stack w [f, g] at partition offsets 0/32/64/96 of a [128, g] array."""
    f, g = w.shape
    out = np.zeros((128, g), np.float32)
    for r in range(4):
        out[32 * r : 32 * r + f] = w
    return out


def _pad_rows(a, dtype):
    """[N, k] -> [NCORES*NSP, k] with each rank's 1250 rows at 1280-stride
    (pad rows zero)."""
    out = np.zeros((NCORES * NSP, a.shape[1]), dtype)
    for r in range(NCORES):
        out[NSP * r : NSP * r + NS] = a[NS * r : NS * r + NS]
    return out


def kernel(**inputs):
    global LAST_RESULTS
    np_fp8 = mybir.dt.np(FP8)
    np_bf16 = mybir.dt.np(BF16)

    x = np.asarray(inputs["x"], dtype=np.float32)
    adj = np.asarray(inputs["adj"], dtype=np.float32)
    W1 = np.asarray(inputs["W1"], np.float32)
    b1 = np.asarray(inputs["b1"], np.float32)

    adjT_q = np.ascontiguousarray(adj.T).astype(np_fp8)
    xT = np.ascontiguousarray(x.T)

    # Host-side s1 = x @ W1 in the padded tile layout.
    s1_pad = _pad_rows(x @ W1, np.float32)
    s1_tiled = np.ascontiguousarray(
        s1_pad.reshape(NT2, 128, F1).transpose(1, 0, 2).reshape(128, NT2 * F1)
    ).astype(np_bf16)

    base = {
        "s1q": s1_tiled,
        "W2": _replicate4(np.asarray(inputs["W2"], np.float32)).astype(np_bf16),
        "W3": _replicate4(np.asarray(inputs["W3"], np.float32)).astype(np_bf16),
        "b1": b1.reshape(F1, 1),
        "b2": np.asarray(inputs["b2"], np.float32).reshape(F2, 1),
        "b3": np.asarray(inputs["b3"], np.float32).reshape(D_OUT, 1),
    }
    in_maps = []
    for c in range(NCORES):
        sl = slice(c * NS, (c + 1) * NS)
        pad = _pad_rows(adjT_q[:, sl], np_fp8)
        adj_tiled = np.ascontiguousarray(
            pad.reshape(NT2, 128, NS).transpose(1, 0, 2).reshape(128, NT2 * NS)
        )
        in_maps.append(
            dict(
                base,
                adjq=adj_tiled,
                xTs=np.ascontiguousarray(xT[:, sl]),
            )
        )

    nc = _get_program()
    trace = bool(int(os.environ.get("GCN_TRACE", "0")))
    extra = {}
    if trace:
        _ensure_ntff_hook()
        if os.environ.get("GCN_TRACE_DIR"):
            os.makedirs(os.environ["GCN_TRACE_DIR"], exist_ok=True)
            extra["tmpdir"] = os.environ["GCN_TRACE_DIR"]
    LAST_RESULTS = run_bass_kernel_spmd(
        nc, in_maps, list(range(NCORES)), trace=trace, **extra
    )
    out = np.concatenate(
        [np.asarray(LAST_RESULTS.results[c]["outT"]).T for c in range(NCORES)],
        axis=0,
    )
    return np.ascontiguousarray(out.astype(np.float32))


# revision 29
# speedup vs baseline: 1.1600x; 1.1026x over previous
"""3-layer GCN (Kipf GraphConvolution) on 8 Trainium2 NeuronCores.

Math per layer: h = relu(adj @ (h @ W) + b); final out = relu(h3 + x).

v4 architecture (row-shard / 1D node partition, fp8-resident adj):
  - adj is transposed on the host, quantized to fp8e4 (e4m3), and core c's
    shard adjT[:, c*NS:(c+1)*NS] is kept RESIDENT in SBUF: one initial load
    instead of streaming bf16 three times. Numerically validated: e4m3(adj)
    adds nothing over the bf16 error floor.
  - PADDED node ordering: the contraction axis uses a virtual ordering of
    8x1280 nodes (each rank's 1250 real nodes + 30 zero rows). Every
    contraction tile is then a full 128-row tile owned by exactly one rank,
    which makes all AllGather buffers plain contiguous slabs (no
    rearranging repacks, no short-tile edge cases). adj/s1 pad rows are
    zeroed on the host; the on-device support stage zeroes its pad rows
    once per layer so no NaN garbage can propagate through 0*x.
  - Support-AllGather: s_local = h_loc.T @ W is computed on the PRODUCING
    core (its h shard is local, ~2us of tiny matmuls BEFORE the
    collective), and the AllGather moves the SUPPORT in tile-major layout.
    The gathered buffer lands directly in agg-lhsT layout with one
    contiguous DMA per rank-block: the entire post-collective
    build/transpose/distribute phase of v2 is gone.
  - Layer-3's 320KB support AllGather is SPLIT into two pipelined meshes;
    the L3 aggregation is reordered to run the first half's tiles while
    the second mesh is still in flight.
  - Layers 1/2 aggregation (fout=20): 4-way PE column tiling, four
    j-tiles' stationaries at tile_position (0, 32g) run concurrently.
  - Layer 3 aggregation (fout=128): 4-way column tiling over feature
    quarters; each stationary streams all 3 chunks (LDWEIGHTS amortized).
  - One tiny warm-up AllGather pays the ncfw first-collective wake-up
    during the adj load; ncfw only begins mesh k when doorbell k+1 lands
    (or ~1us after mesh k-1 ends), so fewer queued warmups = less serial
    mesh time ahead of the real collectives.
"""

import math
import os

import numpy as np

# Experiment: default RDH collective path (no NEURON_RT_DBG_RDH_CC=0).

import concourse.bacc as bacc
import concourse.bass_utils as _bass_utils
import concourse.mybir as mybir
import concourse.tile as tile
from concourse.bass_utils import run_bass_kernel_spmd


# Problem geometry (hardcoded per the harness contract).
N = 10000
D_IN = 128
F1 = 20
F2 = 20
D_OUT = 128
NCORES = 8
NS = N // NCORES  # 1250 real nodes per core
NJ = math.ceil(NS / 128)  # 10 node sub-tiles per core (9 full + 98)
NSTAIL = NS - 128 * (NJ - 1)  # 98
NSP = NJ * 128  # 1280 padded nodes per core
NT2 = NCORES * NJ  # 80 padded contraction tiles
ICHUNK = 512
TILES_PER_SLAB = 10
NSLAB = NT2 // TILES_PER_SLAB  # 8 slabs of 10 tiles (1.6 MB each):
# fewer, fatter DMAs keep the 2 HWDGE rings at HBM line rate.

F32 = mybir.dt.float32
BF16 = mybir.dt.bfloat16
FP8 = mybir.dt.float8e4

# Filled by kernel() so a harness/test can inspect HW timing.
LAST_RESULTS = None


def _chunks(total, step):
    out = []
    i = 0
    while i < total:
        out.append((i, min(step, total - i)))
        i += step
    return out


def build_program(n=N, ncores=NCORES):
    ns = n // ncores
    nt = NT2
    chunks = _chunks(ns, ICHUNK)
    relu = mybir.ActivationFunctionType.Relu
    act_copy = mybir.ActivationFunctionType.Copy

    nc = bacc.Bacc("TRN2", target_bir_lowering=False, debug=False)

    adjq = nc.dram_tensor("adjq", [128, nt * ns], FP8, kind="ExternalInput")
    s1q = nc.dram_tensor("s1q", [128, nt * F1], BF16, kind="ExternalInput")
    W2 = nc.dram_tensor("W2", [128, F2], BF16, kind="ExternalInput")
    W3 = nc.dram_tensor("W3", [128, D_OUT], BF16, kind="ExternalInput")
    b1 = nc.dram_tensor("b1", [F1, 1], F32, kind="ExternalInput")
    b2 = nc.dram_tensor("b2", [F2, 1], F32, kind="ExternalInput")
    b3 = nc.dram_tensor("b3", [D_OUT, 1], F32, kind="ExternalInput")
    xTs = nc.dram_tensor("xTs", [D_IN, ns], F32, kind="ExternalInput")
    outT = nc.dram_tensor("outT", [D_OUT, ns], F32, kind="ExternalOutput")

    with tile.TileContext(nc, num_cores=ncores) as tc:
        with (
            tc.tile_pool(name="const", bufs=1) as const,
            tc.tile_pool(name="adj", bufs=1) as adjpool,
            tc.tile_pool(name="h", bufs=1) as hpool,
            tc.tile_pool(name="red", bufs=2) as redpool,
            tc.tile_pool(name="psy", bufs=1, space="PSUM") as psy_pool,
            tc.tile_pool(name="pss", bufs=1, space="PSUM") as pss_pool,
            tc.tile_pool(name="dram", bufs=1, space="DRAM") as dpool,
        ):
            w2_sb = const.tile([128, F2], BF16, tag="w2")
            w3_sb = const.tile([128, D_OUT], BF16, tag="w3")
            b1_sb = const.tile([F1, 1], F32, tag="b1")
            b2_sb = const.tile([F2, 1], F32, tag="b2")
            b3_sb = const.tile([D_OUT, 1], F32, tag="b3")
            xts_sb = const.tile([D_IN, ns], F32, tag="xts")
            s1_sb = const.tile([128, nt * F1], BF16, tag="s1")
            # One tiny warm-up AllGather, triggered at t~1-2us (gpsimd
            # carries nothing else up front). Its mesh is kicked by the
            # first real collective's doorbell; more warmups would only
            # queue more serial mesh work ahead of the real ones.
            warm_in = dpool.tile([1, 4], F32, tag="warmin")
            warm_out = dpool.tile([ncores, 4], F32, tag="warmout",
                                  addr_space="Shared")
            nc.sync.dma_start(out=warm_in[:, :], in_=xTs[0:1, 0:4])
            nc.gpsimd.collective_compute(
                "AllGather",
                mybir.AluOpType.bypass,
                replica_groups=[list(range(ncores))],
                ins=[warm_in.opt()],
                outs=[warm_out.opt()],
            )

            # Const loads on the HWDGE rings (NOT gpsimd: SWDGE descriptor
            # generation for these was delaying the warm-up trigger ~18us).
            nc.sync.dma_start(out=s1_sb[:, :], in_=s1q[:, :])
            for sb, dr in (
                (w2_sb, W2),
                (w3_sb, W3),
                (b1_sb, b1),
                (b2_sb, b2),
                (b3_sb, b3),
            ):
                nc.scalar.dma_start(out=sb[:, :], in_=dr[:, :])

            # ---- resident adj: 16 slab loads alternating the 2 HWDGE rings
            slabs = []
            for s in range(NSLAB):
                at = adjpool.tile([128, TILES_PER_SLAB * ns], FP8,
                                  tag=f"slab{s}")
                eng = nc.sync if s % 2 == 0 else nc.scalar
                c0 = s * TILES_PER_SLAB * ns
                eng.dma_start(
                    out=at[:, :], in_=adjq[:, c0 : c0 + TILES_PER_SLAB * ns]
                )
                slabs.append(at)

            def adj_slice(t, i0, ilen):
                s, r = divmod(t, TILES_PER_SLAB)
                return slabs[s][:, r * ns + i0 : r * ns + i0 + ilen]

            # ---- col-tiled aggregation for fout<=32 (layers 1 and 2).
            def agg_tiled(s_tiles, fout, lname, chunk_major=False):
                lastt = [max(t for t in range(nt) if t % 4 == g)
                         for g in range(4)]
                psy = [
                    psy_pool.tile([128, ICHUNK], F32, tag=f"psy{ic}",
                                  name=f"psy_{lname}_{ic}")
                    for ic in range(len(chunks))
                ]
                if chunk_major:
                    order = [(ic, t) for ic in range(len(chunks))
                             for t in range(nt)]
                else:
                    order = [(ic, b0 + g) for b0 in range(0, nt, 4)
                             for ic in range(len(chunks)) for g in range(4)
                             if b0 + g < nt]
                for ic, t in order:
                    i0, ilen = chunks[ic]
                    g = t % 4
                    nc.tensor.matmul(
                        psy[ic][32 * g : 32 * g + fout, :ilen],
                        lhsT=s_tiles[t][:, :fout],
                        rhs=adj_slice(t, i0, ilen),
                        start=(t < 4),
                        stop=(t == lastt[g]),
                        tile_position=(0, 32 * g),
                    )
                return psy

            def reduce_relu(psy, fout, dst, b_sb):
                """dst[:, i0:i0+ilen] = relu(sum of 4 col-group slices + b).

                Each tensor op may read at most ONE PSUM operand: ACT
                evacuates group 0 while DVE chains the other three."""
                for ic, (i0, ilen) in enumerate(chunks):
                    a = redpool.tile([F1, ICHUNK], F32, tag="ra",
                                     name=f"ra{ic}")
                    b = redpool.tile([F1, ICHUNK], F32, tag="rb",
                                     name=f"rb{ic}")
                    nc.scalar.activation(a[:fout, :ilen],
                                         psy[ic][0:fout, :ilen], act_copy)
                    nc.vector.tensor_add(b[:fout, :ilen], a[:fout, :ilen],
                                         psy[ic][32 : 32 + fout, :ilen])
                    nc.vector.tensor_add(a[:fout, :ilen], b[:fout, :ilen],
                                         psy[ic][64 : 64 + fout, :ilen])
                    nc.vector.tensor_add(b[:fout, :ilen], a[:fout, :ilen],
                                         psy[ic][96 : 96 + fout, :ilen])
                    nc.scalar.activation(dst[:fout, i0 : i0 + ilen],
                                         b[:fout, :ilen], relu,
                                         bias=b_sb[:fout, :])

            def support_allgather(h_loc, w_sb, fin, fout, layer, parts):
                """s_local = h_loc.T @ W on the PRODUCING core, AllGather
                the SUPPORT in tile-major layout (all buffers contiguous
                thanks to the 1280-padded node ordering). `parts` splits the
                local j-range into multiple pipelined collectives; returns
                (s_tiles, order) where order lists tile indices in
                arrival order."""
                s_stage = hpool.tile([128, NJ * fout], BF16,
                                     tag=f"sstage{layer}")
                # Zero the 30 pad rows of the last j-block once: pad adj
                # rows are zero but 0*NaN = NaN, so the stage must hold
                # finite values there. (Partition base must be 32-aligned;
                # rows 96..97 are rewritten by the j=9 copy below.)
                nc.vector.memset(s_stage[96:, (NJ - 1) * fout :], 0.0)
                for j in range(NJ):
                    mj = min(128, ns - j * 128)
                    ps = pss_pool.tile([128, 512], F32, tag=f"psl{j % 2}",
                                       name=f"psl_{layer}_{j}")
                    nc.tensor.matmul(
                        ps[:mj, :fout],
                        lhsT=h_loc[:fin, j * 128 : j * 128 + mj],
                        rhs=w_sb[:fin, :fout],
                        start=True,
                        stop=True,
                    )
                    if j % 2 == 0:
                        nc.vector.tensor_copy(
                            s_stage[:mj, j * fout : (j + 1) * fout],
                            ps[:mj, :fout],
                        )
                    else:
                        nc.scalar.activation(
                            s_stage[:mj, j * fout : (j + 1) * fout],
                            ps[:mj, :fout],
                            act_copy,
                        )
                s_full = hpool.tile([128, nt * fout], BF16,
                                    tag=f"sfull{layer}")
                order = []
                # All bounce-in DMAs and doorbells FIRST: a landing DMA
                # waits on its mesh, and HWDGE rings are FIFO per engine,
                # so a later part's cc_in must not queue behind an earlier
                # part's landing.
                ccs = []
                for pi, (j0, j1) in enumerate(parts):
                    w = j1 - j0
                    cc_in = dpool.tile([128, w * fout], BF16,
                                       tag=f"ccin{layer}_{pi}")
                    cc_out = dpool.tile([128 * ncores, w * fout], BF16,
                                        tag=f"ccout{layer}_{pi}",
                                        addr_space="Shared")
                    nc.sync.dma_start(
                        out=cc_in[:, :],
                        in_=s_stage[:, j0 * fout : j1 * fout],
                    )
                    ccs.append((j0, j1, cc_in, cc_out))
                for j0, j1, cc_in, cc_out in ccs:
                    nc.gpsimd.collective_compute(
                        "AllGather",
                        mybir.AluOpType.bypass,
                        replica_groups=[list(range(ncores))],
                        ins=[cc_in.opt()],
                        outs=[cc_out.opt()],
                    )
                for j0, j1, cc_in, cc_out in ccs:
                    # Land rank-blocks into tile-major s_full: tile index of
                    # (rank r, local j) is r*NJ + j. Per-partition runs are
                    # w*fout contiguous elements -> clean fat descriptors.
                    # Rank 0 lands in its own small DMA so the next agg's
                    # first tiles unblock ~1us sooner.
                    for r0, r1, eng in ((0, 1, nc.sync), (1, 4, nc.sync),
                                        (4, 8, nc.scalar)):
                        eng.dma_start(
                            out=s_full[:, :].rearrange(
                                "p (r x) -> p r x", r=ncores
                            )[:, r0:r1, j0 * fout : j1 * fout],
                            in_=cc_out[:, :].rearrange(
                                "(r p) x -> p r x", p=128
                            )[:, r0:r1, :],
                        )
                    order.extend(r * NJ + j for r in range(ncores)
                                 for j in range(j0, j1))
                tiles = [s_full[:, t * fout : (t + 1) * fout]
                         for t in range(nt)]
                return tiles, order

            # ---- Layer 1: s1 from host (padded tile order) ----
            # chunk-major: PSUM bank 0 completes ~2us before banks 1/2 at
            # the end of the (DMA-paced) load, so relu chunk 0 and the
            # local-support matmuls start earlier.
            s1_tiles = [s1_sb[:, t * F1 : (t + 1) * F1] for t in range(nt)]
            psy1 = agg_tiled(s1_tiles, F1, "l1", chunk_major=True)
            h1_loc = hpool.tile([F1, ns], BF16, tag="hloc1")
            reduce_relu(psy1, F1, h1_loc, b1_sb)
            s2_tiles, _ = support_allgather(h1_loc, w2_sb, F1, F2, 2,
                                            parts=[(0, NJ)])

            # ---- Layer 2 ----
            psy2 = agg_tiled(s2_tiles, F2, "l2", chunk_major=True)
            h2_loc = hpool.tile([F2, ns], BF16, tag="hloc2")
            reduce_relu(psy2, F2, h2_loc, b2_sb)
            s3_tiles, l3_order = support_allgather(
                h2_loc, w3_sb, F2, D_OUT, 3, parts=[(0, NJ // 2), (NJ // 2, NJ)]
            )

            # ---- Layer 3 (fout=128, 4-way PE column tiling over feature
            # quarters; tiles processed in AllGather-arrival order so the
            # first half aggregates while the second mesh is in flight) ----
            psy3 = [
                psy_pool.tile([128, ICHUNK], F32, tag=f"psy{ic}",
                              name=f"psy3_{ic}")
                for ic in range(len(chunks))
            ]
            for idx, t in enumerate(l3_order):
                for ic, (i0, ilen) in enumerate(chunks):
                    for g in range(4):
                        nc.tensor.matmul(
                            psy3[ic][32 * g : 32 * g + 32, :ilen],
                            lhsT=s3_tiles[t][:, 32 * g : 32 * g + 32],
                            rhs=adj_slice(t, i0, ilen),
                            start=(idx == 0),
                            stop=(idx == nt - 1),
                            tile_position=(0, 32 * g),
                        )

            # xts is only needed by the epilogue below; load it late so it
            # doesn't compete with adj/s1 for HBM bandwidth at kernel start.
            nc.scalar.dma_start(out=xts_sb[:, :], in_=xTs[:, :])

            # ---- out = relu(h3 + x) ----
            o_sb = hpool.tile([D_OUT, ns], F32, tag="ostage")
            for ic, (i0, ilen) in enumerate(chunks):
                a = redpool.tile([D_OUT, ICHUNK], F32, tag="fa", name=f"fa{ic}")
                nc.vector.tensor_add(a[:, :ilen], xts_sb[:, i0 : i0 + ilen],
                                     psy3[ic][:, :ilen])
                nc.scalar.activation(o_sb[:, i0 : i0 + ilen], a[:, :ilen], relu,
                                     bias=b3_sb[:, :])
                nc.sync.dma_start(out=outT[:, i0 : i0 + ilen],
                                  in_=o_sb[:, i0 : i0 + ilen])

    nc.compile()
    return nc


def _ensure_ntff_hook():
    """Register the axon NTFF profile hook if the image's antenv lacks it."""
    import contextlib
    import ctypes
    import sys
    import types

    try:
        from antenv.axon_hooks import get_axon_ntff_profile_hook  # noqa: F401

        return
    except ImportError:
        pass

    so_path = "/opt/axon/libaxon_pjrt.so"
    lib = ctypes.CDLL(so_path)
    if not hasattr(lib, "axon_start_nrt_profile"):
        return
    lib.axon_start_nrt_profile.argtypes = [
        ctypes.POINTER(ctypes.c_int64),
        ctypes.c_size_t,
    ]
    lib.axon_start_nrt_profile.restype = ctypes.c_int64
    lib.axon_stop_nrt_profile.argtypes = [ctypes.c_char_p]
    lib.axon_stop_nrt_profile.restype = ctypes.c_int64

    @contextlib.contextmanager
    def _hook(output_dir, device_ids):
        import jax

        jax.devices()
        if device_ids:
            ids = (ctypes.c_int64 * len(device_ids))(*device_ids)
            rc = lib.axon_start_nrt_profile(ids, len(device_ids))
        else:
            rc = lib.axon_start_nrt_profile(None, 0)
        if rc != 0:
            raise RuntimeError(f"axon_start_nrt_profile rc={rc}")
        try:
            yield
        finally:
            n = lib.axon_stop_nrt_profile(str(output_dir).encode())
            print(f"ntff profile: {n} file(s) written to {output_dir}")

    mod = types.ModuleType("antenv.axon_hooks")
    _state = {"hook": _hook}
    mod.get_axon_ntff_profile_hook = lambda: _state["hook"]
    mod.set_axon_ntff_profile_hook = lambda h: _state.update(hook=h)
    sys.modules["antenv.axon_hooks"] = mod
    import antenv

    antenv.axon_hooks = mod


_PROGRAM = None


def _get_program():
    global _PROGRAM
    if _PROGRAM is None:
        _PROGRAM = build_program()
    return _PROGRAM


def _replicate4(w):
    """Stack w [f, g] at partition offsets 0/32/64/96 of a [128, g] array."""
    f, g = w.shape
    out = np.zeros((128, g), np.float32)
    for r in range(4):
        out[32 * r : 32 * r + f] = w
    return out


def _pad_rows(a, dtype):
    """[N, k] -> [NCORES*NSP, k] with each rank's 1250 rows at 1280-stride
    (pad rows zero)."""
    out = np.zeros((NCORES * NSP, a.shape[1]), dtype)
    for r in range(NCORES):
        out[NSP * r : NSP * r + NS] = a[NS * r : NS * r + NS]
    return out


def kernel(**inputs):
    global LAST_RESULTS
    np_fp8 = mybir.dt.np(FP8)
    np_bf16 = mybir.dt.np(BF16)

    x = np.asarray(inputs["x"], dtype=np.float32)
    adj = np.asarray(inputs["adj"], dtype=np.float32)
    W1 = np.asarray(inputs["W1"], np.float32)
    b1 = np.asarray(inputs["b1"], np.float32)

    adjT_q = np.ascontiguousarray(adj.T).astype(np_fp8)
    xT = np.ascontiguousarray(x.T)

    # Host-side s1 = x @ W1 in the padded tile layout.
    s1_pad = _pad_rows(x @ W1, np.float32)
    s1_tiled = np.ascontiguousarray(
        s1_pad.reshape(NT2, 128, F1).transpose(1, 0, 2).reshape(128, NT2 * F1)
    ).astype(np_bf16)

    base = {
        "s1q": s1_tiled,
        "W2": _replicate4(np.asarray(inputs["W2"], np.float32)).astype(np_bf16),
        "W3": _replicate4(np.asarray(inputs["W3"], np.float32)).astype(np_bf16),
        "b1": b1.reshape(F1, 1),
        "b2": np.asarray(inputs["b2"], np.float32).reshape(F2, 1),
        "b3": np.asarray(inputs["b3"], np.float32).reshape(D_OUT, 1),
    }
    in_maps = []
    for c in range(NCORES):
        sl = slice(c * NS, (c + 1) * NS)
        pad = _pad_rows(adjT_q[:, sl], np_fp8)
        adj_tiled = np.ascontiguousarray(
            pad.reshape(NT2, 128, NS).transpose(1, 0, 2).reshape(128, NT2 * NS)
        )
        in_maps.append(
            dict(
                base,
                adjq=adj_tiled,
                xTs=np.ascontiguousarray(xT[:, sl]),
            )
        )

    nc = _get_program()
    trace = bool(int(os.environ.get("GCN_TRACE", "0")))
    extra = {}
    if trace:
        _ensure_ntff_hook()
        if os.environ.get("GCN_TRACE_DIR"):
            os.makedirs(os.environ["GCN_TRACE_DIR"], exist_ok=True)
            extra["tmpdir"] = os.environ["GCN_TRACE_DIR"]
    LAST_RESULTS = run_bass_kernel_spmd(
        nc, in_maps, list(range(NCORES)), trace=trace, **extra
    )
    out = np.concatenate(
        [np.asarray(LAST_RESULTS.results[c]["outT"]).T for c in range(NCORES)],
        axis=0,
    )
    return np.ascontiguousarray(out.astype(np.float32))


# revision 30
# speedup vs baseline: 1.1832x; 1.0199x over previous
"""3-layer GCN (Kipf GraphConvolution) on 8 Trainium2 NeuronCores.

Math per layer: h = relu(adj @ (h @ W) + b); final out = relu(h3 + x).

v4 architecture (row-shard / 1D node partition, fp8-resident adj):
  - adj is transposed on the host, quantized to fp8e4 (e4m3), and core c's
    shard adjT[:, c*NS:(c+1)*NS] is kept RESIDENT in SBUF: one initial load
    instead of streaming bf16 three times. Numerically validated: e4m3(adj)
    adds nothing over the bf16 error floor.
  - PADDED node ordering: the contraction axis uses a virtual ordering of
    8x1280 nodes (each rank's 1250 real nodes + 30 zero rows). Every
    contraction tile is then a full 128-row tile owned by exactly one rank,
    which makes all AllGather buffers plain contiguous slabs (no
    rearranging repacks, no short-tile edge cases). adj/s1 pad rows are
    zeroed on the host; the on-device support stage zeroes its pad rows
    once per layer so no NaN garbage can propagate through 0*x.
  - Support-AllGather: s_local = h_loc.T @ W is computed on the PRODUCING
    core (its h shard is local, ~2us of tiny matmuls BEFORE the
    collective), and the AllGather moves the SUPPORT in tile-major layout.
    The gathered buffer lands directly in agg-lhsT layout with one
    contiguous DMA per rank-block: the entire post-collective
    build/transpose/distribute phase of v2 is gone.
  - Layer-3's 320KB support AllGather is SPLIT into two pipelined meshes;
    the L3 aggregation is reordered to run the first half's tiles while
    the second mesh is still in flight.
  - Layers 1/2 aggregation (fout=20): 4-way PE column tiling, four
    j-tiles' stationaries at tile_position (0, 32g) run concurrently.
  - Layer 3 aggregation (fout=128): 4-way column tiling over feature
    quarters; each stationary streams all 3 chunks (LDWEIGHTS amortized).
  - One tiny warm-up AllGather pays the ncfw first-collective wake-up
    during the adj load; ncfw only begins mesh k when doorbell k+1 lands
    (or ~1us after mesh k-1 ends), so fewer queued warmups = less serial
    mesh time ahead of the real collectives.
"""

import math
import os

import numpy as np

# Experiment: default RDH collective path (no NEURON_RT_DBG_RDH_CC=0).

import concourse.bacc as bacc
import concourse.bass_utils as _bass_utils
import concourse.mybir as mybir
import concourse.tile as tile
from concourse.bass_utils import run_bass_kernel_spmd


# Problem geometry (hardcoded per the harness contract).
N = 10000
D_IN = 128
F1 = 20
F2 = 20
D_OUT = 128
NCORES = 8
NS = N // NCORES  # 1250 real nodes per core
NJ = math.ceil(NS / 128)  # 10 node sub-tiles per core (9 full + 98)
NSTAIL = NS - 128 * (NJ - 1)  # 98
NSP = NJ * 128  # 1280 padded nodes per core
NT2 = NCORES * NJ  # 80 padded contraction tiles
ICHUNK = 512
TILES_PER_SLAB = 10
NSLAB = NT2 // TILES_PER_SLAB  # 8 slabs of 10 tiles (1.6 MB each):
# fewer, fatter DMAs keep the 2 HWDGE rings at HBM line rate.

F32 = mybir.dt.float32
BF16 = mybir.dt.bfloat16
FP8 = mybir.dt.float8e4

# Filled by kernel() so a harness/test can inspect HW timing.
LAST_RESULTS = None


def _chunks(total, step):
    out = []
    i = 0
    while i < total:
        out.append((i, min(step, total - i)))
        i += step
    return out


def build_program(n=N, ncores=NCORES):
    ns = n // ncores
    nt = NT2
    chunks = _chunks(ns, ICHUNK)
    relu = mybir.ActivationFunctionType.Relu
    act_copy = mybir.ActivationFunctionType.Copy

    nc = bacc.Bacc("TRN2", target_bir_lowering=False, debug=False)

    adjq = nc.dram_tensor("adjq", [128, nt * ns], FP8, kind="ExternalInput")
    s1q = nc.dram_tensor("s1q", [128, nt * F1], BF16, kind="ExternalInput")
    W2 = nc.dram_tensor("W2", [128, F2], BF16, kind="ExternalInput")
    W3 = nc.dram_tensor("W3", [128, D_OUT], BF16, kind="ExternalInput")
    b1 = nc.dram_tensor("b1", [F1, 1], F32, kind="ExternalInput")
    b2 = nc.dram_tensor("b2", [F2, 1], F32, kind="ExternalInput")
    b3 = nc.dram_tensor("b3", [D_OUT, 1], F32, kind="ExternalInput")
    xTs = nc.dram_tensor("xTs", [D_IN, ns], F32, kind="ExternalInput")
    outT = nc.dram_tensor("outT", [D_OUT, ns], F32, kind="ExternalOutput")

    with tile.TileContext(nc, num_cores=ncores) as tc:
        with (
            tc.tile_pool(name="const", bufs=1) as const,
            tc.tile_pool(name="adj", bufs=1) as adjpool,
            tc.tile_pool(name="h", bufs=1) as hpool,
            tc.tile_pool(name="red", bufs=2) as redpool,
            tc.tile_pool(name="psy", bufs=1, space="PSUM") as psy_pool,
            tc.tile_pool(name="pss", bufs=1, space="PSUM") as pss_pool,
            tc.tile_pool(name="dram", bufs=1, space="DRAM") as dpool,
        ):
            w2_sb = const.tile([128, F2], BF16, tag="w2")
            w3_sb = const.tile([128, D_OUT], BF16, tag="w3")
            b1_sb = const.tile([F1, 1], F32, tag="b1")
            b2_sb = const.tile([F2, 1], F32, tag="b2")
            b3_sb = const.tile([D_OUT, 1], F32, tag="b3")
            xts_sb = const.tile([D_IN, ns], F32, tag="xts")
            s1_sb = const.tile([128, nt * F1], BF16, tag="s1")
            # One tiny warm-up AllGather, triggered at t~1-2us (gpsimd
            # carries nothing else up front). Its mesh is kicked by the
            # first real collective's doorbell; more warmups would only
            # queue more serial mesh work ahead of the real ones.
            warm_in = dpool.tile([1, 4], F32, tag="warmin")
            warm_out = dpool.tile([ncores, 4], F32, tag="warmout",
                                  addr_space="Shared")
            nc.sync.dma_start(out=warm_in[:, :], in_=xTs[0:1, 0:4])
            nc.gpsimd.collective_compute(
                "AllGather",
                mybir.AluOpType.bypass,
                replica_groups=[list(range(ncores))],
                ins=[warm_in.opt()],
                outs=[warm_out.opt()],
            )

            # Const loads on the HWDGE rings (NOT gpsimd: SWDGE descriptor
            # generation for these was delaying the warm-up trigger ~18us).
            nc.sync.dma_start(out=s1_sb[:, :], in_=s1q[:, :])
            for sb, dr in (
                (w2_sb, W2),
                (w3_sb, W3),
                (b1_sb, b1),
                (b2_sb, b2),
                (b3_sb, b3),
            ):
                nc.scalar.dma_start(out=sb[:, :], in_=dr[:, :])

            # ---- resident adj: 16 slab loads alternating the 2 HWDGE rings
            slabs = []
            for s in range(NSLAB):
                at = adjpool.tile([128, TILES_PER_SLAB * ns], FP8,
                                  tag=f"slab{s}")
                eng = nc.sync if s % 2 == 0 else nc.scalar
                c0 = s * TILES_PER_SLAB * ns
                eng.dma_start(
                    out=at[:, :], in_=adjq[:, c0 : c0 + TILES_PER_SLAB * ns]
                )
                slabs.append(at)

            def adj_slice(t, i0, ilen):
                s, r = divmod(t, TILES_PER_SLAB)
                return slabs[s][:, r * ns + i0 : r * ns + i0 + ilen]

            # ---- col-tiled aggregation for fout<=32 (layers 1 and 2).
            def agg_tiled(s_tiles, fout, lname, chunk_major=False):
                lastt = [max(t for t in range(nt) if t % 4 == g)
                         for g in range(4)]
                psy = [
                    psy_pool.tile([128, ICHUNK], F32, tag=f"psy{ic}",
                                  name=f"psy_{lname}_{ic}")
                    for ic in range(len(chunks))
                ]
                if chunk_major:
                    order = [(ic, t) for ic in range(len(chunks))
                             for t in range(nt)]
                else:
                    order = [(ic, b0 + g) for b0 in range(0, nt, 4)
                             for ic in range(len(chunks)) for g in range(4)
                             if b0 + g < nt]
                for ic, t in order:
                    i0, ilen = chunks[ic]
                    g = t % 4
                    nc.tensor.matmul(
                        psy[ic][32 * g : 32 * g + fout, :ilen],
                        lhsT=s_tiles[t][:, :fout],
                        rhs=adj_slice(t, i0, ilen),
                        start=(t < 4),
                        stop=(t == lastt[g]),
                        tile_position=(0, 32 * g),
                    )
                return psy

            def reduce_relu(psy, fout, dst, b_sb):
                """dst[:, i0:i0+ilen] = relu(sum of 4 col-group slices + b).

                Each tensor op may read at most ONE PSUM operand: ACT
                evacuates group 0 while DVE chains the other three."""
                for ic, (i0, ilen) in enumerate(chunks):
                    a = redpool.tile([F1, ICHUNK], F32, tag="ra",
                                     name=f"ra{ic}")
                    b = redpool.tile([F1, ICHUNK], F32, tag="rb",
                                     name=f"rb{ic}")
                    nc.scalar.activation(a[:fout, :ilen],
                                         psy[ic][0:fout, :ilen], act_copy)
                    nc.vector.tensor_add(b[:fout, :ilen], a[:fout, :ilen],
                                         psy[ic][32 : 32 + fout, :ilen])
                    nc.vector.tensor_add(a[:fout, :ilen], b[:fout, :ilen],
                                         psy[ic][64 : 64 + fout, :ilen])
                    nc.vector.tensor_add(b[:fout, :ilen], a[:fout, :ilen],
                                         psy[ic][96 : 96 + fout, :ilen])
                    nc.scalar.activation(dst[:fout, i0 : i0 + ilen],
                                         b[:fout, :ilen], relu,
                                         bias=b_sb[:fout, :])

            def support_allgather(h_loc, w_sb, fin, fout, layer, parts):
                """s_local = h_loc.T @ W on the PRODUCING core, AllGather
                the SUPPORT in tile-major layout (all buffers contiguous
                thanks to the 1280-padded node ordering). `parts` splits the
                local j-range into multiple pipelined collectives; returns
                (s_tiles, order) where order lists tile indices in
                arrival order."""
                s_stage = hpool.tile([128, NJ * fout], BF16,
                                     tag=f"sstage{layer}")
                # Zero the 30 pad rows of the last j-block once: pad adj
                # rows are zero but 0*NaN = NaN, so the stage must hold
                # finite values there. (Partition base must be 32-aligned;
                # rows 96..97 are rewritten by the j=9 copy below.)
                nc.vector.memset(s_stage[96:, (NJ - 1) * fout :], 0.0)
                for j in range(NJ):
                    mj = min(128, ns - j * 128)
                    ps = pss_pool.tile([128, 512], F32, tag=f"psl{j % 2}",
                                       name=f"psl_{layer}_{j}")
                    nc.tensor.matmul(
                        ps[:mj, :fout],
                        lhsT=h_loc[:fin, j * 128 : j * 128 + mj],
                        rhs=w_sb[:fin, :fout],
                        start=True,
                        stop=True,
                    )
                    if j % 2 == 0:
                        nc.vector.tensor_copy(
                            s_stage[:mj, j * fout : (j + 1) * fout],
                            ps[:mj, :fout],
                        )
                    else:
                        nc.scalar.activation(
                            s_stage[:mj, j * fout : (j + 1) * fout],
                            ps[:mj, :fout],
                            act_copy,
                        )
                s_full = hpool.tile([128, nt * fout], BF16,
                                    tag=f"sfull{layer}")
                order = []
                # All bounce-in DMAs and doorbells FIRST: a landing DMA
                # waits on its mesh, and HWDGE rings are FIFO per engine,
                # so a later part's cc_in must not queue behind an earlier
                # part's landing.
                ccs = []
                for pi, (j0, j1) in enumerate(parts):
                    w = j1 - j0
                    cc_in = dpool.tile([128, w * fout], BF16,
                                       tag=f"ccin{layer}_{pi}")
                    cc_out = dpool.tile([128 * ncores, w * fout], BF16,
                                        tag=f"ccout{layer}_{pi}",
                                        addr_space="Shared")
                    nc.sync.dma_start(
                        out=cc_in[:, :],
                        in_=s_stage[:, j0 * fout : j1 * fout],
                    )
                    ccs.append((j0, j1, cc_in, cc_out))
                for j0, j1, cc_in, cc_out in ccs:
                    nc.gpsimd.collective_compute(
                        "AllGather",
                        mybir.AluOpType.bypass,
                        replica_groups=[list(range(ncores))],
                        ins=[cc_in.opt()],
                        outs=[cc_out.opt()],
                    )
                for j0, j1, cc_in, cc_out in ccs:
                    # Land rank-blocks into tile-major s_full: tile index of
                    # (rank r, local j) is r*NJ + j. Per-partition runs are
                    # w*fout contiguous elements -> clean fat descriptors.
                    for r0, r1, eng in ((0, 4, nc.sync), (4, 8, nc.scalar)):
                        eng.dma_start(
                            out=s_full[:, :].rearrange(
                                "p (r x) -> p r x", r=ncores
                            )[:, r0:r1, j0 * fout : j1 * fout],
                            in_=cc_out[:, :].rearrange(
                                "(r p) x -> p r x", p=128
                            )[:, r0:r1, :],
                        )
                    order.extend(r * NJ + j for r in range(ncores)
                                 for j in range(j0, j1))
                tiles = [s_full[:, t * fout : (t + 1) * fout]
                         for t in range(nt)]
                return tiles, order

            # ---- Layer 1: s1 from host (padded tile order) ----
            s1_tiles = [s1_sb[:, t * F1 : (t + 1) * F1] for t in range(nt)]
            psy1 = agg_tiled(s1_tiles, F1, "l1")
            h1_loc = hpool.tile([F1, ns], BF16, tag="hloc1")
            reduce_relu(psy1, F1, h1_loc, b1_sb)
            s2_tiles, _ = support_allgather(h1_loc, w2_sb, F1, F2, 2,
                                            parts=[(0, NJ)])

            # ---- Layer 2 ----
            psy2 = agg_tiled(s2_tiles, F2, "l2", chunk_major=True)
            h2_loc = hpool.tile([F2, ns], BF16, tag="hloc2")
            reduce_relu(psy2, F2, h2_loc, b2_sb)
            s3_tiles, l3_order = support_allgather(
                h2_loc, w3_sb, F2, D_OUT, 3, parts=[(0, NJ // 2), (NJ // 2, NJ)]
            )

            # ---- Layer 3 (fout=128, 4-way PE column tiling over feature
            # quarters; tiles processed in AllGather-arrival order so the
            # first half aggregates while the second mesh is in flight) ----
            psy3 = [
                psy_pool.tile([128, ICHUNK], F32, tag=f"psy{ic}",
                              name=f"psy3_{ic}")
                for ic in range(len(chunks))
            ]
            for idx, t in enumerate(l3_order):
                for ic, (i0, ilen) in enumerate(chunks):
                    for g in range(4):
                        nc.tensor.matmul(
                            psy3[ic][32 * g : 32 * g + 32, :ilen],
                            lhsT=s3_tiles[t][:, 32 * g : 32 * g + 32],
                            rhs=adj_slice(t, i0, ilen),
                            start=(idx == 0),
                            stop=(idx == nt - 1),
                            tile_position=(0, 32 * g),
                        )

            # xts is only needed by the epilogue below; load it late so it
            # doesn't compete with adj/s1 for HBM bandwidth at kernel start.
            nc.scalar.dma_start(out=xts_sb[:, :], in_=xTs[:, :])

            # ---- out = relu(h3 + x) ----
            o_sb = hpool.tile([D_OUT, ns], F32, tag="ostage")
            for ic, (i0, ilen) in enumerate(chunks):
                a = redpool.tile([D_OUT, ICHUNK], F32, tag="fa", name=f"fa{ic}")
                nc.vector.tensor_add(a[:, :ilen], xts_sb[:, i0 : i0 + ilen],
                                     psy3[ic][:, :ilen])
                nc.scalar.activation(o_sb[:, i0 : i0 + ilen], a[:, :ilen], relu,
                                     bias=b3_sb[:, :])
            nc.sync.dma_start(out=outT[:, :], in_=o_sb[:, :])

    nc.compile()
    return nc


def _ensure_ntff_hook():
    """Register the axon NTFF profile hook if the image's antenv lacks it."""
    import contextlib
    import ctypes
    import sys
    import types

    try:
        from antenv.axon_hooks import get_axon_ntff_profile_hook  # noqa: F401

        return
    except ImportError:
        pass

    so_path = "/opt/axon/libaxon_pjrt.so"
    lib = ctypes.CDLL(so_path)
    if not hasattr(lib, "axon_start_nrt_profile"):
        return
    lib.axon_start_nrt_profile.argtypes = [
        ctypes.POINTER(ctypes.c_int64),
        ctypes.c_size_t,
    ]
    lib.axon_start_nrt_profile.restype = ctypes.c_int64
    lib.axon_stop_nrt_profile.argtypes = [ctypes.c_char_p]
    lib.axon_stop_nrt_profile.restype = ctypes.c_int64

    @contextlib.contextmanager
    def _hook(output_dir, device_ids):
        import jax

        jax.devices()
        if device_ids:
            ids = (ctypes.c_int64 * len(device_ids))(*device_ids)
            rc = lib.axon_start_nrt_profile(ids, len(device_ids))
        else:
            rc = lib.axon_start_nrt_profile(None, 0)
        if rc != 0:
            raise RuntimeError(f"axon_start_nrt_profile rc={rc}")
        try:
            yield
        finally:
            n = lib.axon_stop_nrt_profile(str(output_dir).encode())
            print(f"ntff profile: {n} file(s) written to {output_dir}")

    mod = types.ModuleType("antenv.axon_hooks")
    _state = {"hook": _hook}
    mod.get_axon_ntff_profile_hook = lambda: _state["hook"]
    mod.set_axon_ntff_profile_hook = lambda h: _state.update(hook=h)
    sys.modules["antenv.axon_hooks"] = mod
    import antenv

    antenv.axon_hooks = mod


_PROGRAM = None


def _get_program():
    global _PROGRAM
    if _PROGRAM is None:
        _PROGRAM = build_program()
    return _PROGRAM


def _replicate4(w):
    """Stack w [f, g] at partition offsets 0/32/64/96 of a [128, g] array."""
    f, g = w.shape
    out = np.zeros((128, g), np.float32)
    for r in range(4):
        out[32 * r : 32 * r + f] = w
    return out


def _pad_rows(a, dtype):
    """[N, k] -> [NCORES*NSP, k] with each rank's 1250 rows at 1280-stride
    (pad rows zero)."""
    out = np.zeros((NCORES * NSP, a.shape[1]), dtype)
    for r in range(NCORES):
        out[NSP * r : NSP * r + NS] = a[NS * r : NS * r + NS]
    return out


def kernel(**inputs):
    global LAST_RESULTS
    np_fp8 = mybir.dt.np(FP8)
    np_bf16 = mybir.dt.np(BF16)

    x = np.asarray(inputs["x"], dtype=np.float32)
    adj = np.asarray(inputs["adj"], dtype=np.float32)
    W1 = np.asarray(inputs["W1"], np.float32)
    b1 = np.asarray(inputs["b1"], np.float32)

    adjT_q = np.ascontiguousarray(adj.T).astype(np_fp8)
    xT = np.ascontiguousarray(x.T)

    # Host-side s1 = x @ W1 in the padded tile layout.
    s1_pad = _pad_rows(x @ W1, np.float32)
    s1_tiled = np.ascontiguousarray(
        s1_pad.reshape(NT2, 128, F1).transpose(1, 0, 2).reshape(128, NT2 * F1)
    ).astype(np_bf16)

    base = {
        "s1q": s1_tiled,
        "W2": _replicate4(np.asarray(inputs["W2"], np.float32)).astype(np_bf16),
        "W3": _replicate4(np.asarray(inputs["W3"], np.float32)).astype(np_bf16),
        "b1": b1.reshape(F1, 1),
        "b2": np.asarray(inputs["b2"], np.float32).reshape(F2, 1),
        "b3": np.asarray(inputs["b3"], np.float32).reshape(D_OUT, 1),
    }
    in_maps = []
    for c in range(NCORES):
        sl = slice(c * NS, (c + 1) * NS)
        pad = _pad_rows(adjT_q[:, sl], np_fp8)
        adj_tiled = np.ascontiguousarray(
            pad.reshape(NT2, 128, NS).transpose(1, 0, 2).reshape(128, NT2 * NS)
        )
        in_maps.append(
            dict(
                base,
                adjq=adj_tiled,
                xTs=np.ascontiguousarray(xT[:, sl]),
            )
        )

    nc = _get_program()
    trace = bool(int(os.environ.get("GCN_TRACE", "0")))
    extra = {}
    if trace:
        _ensure_ntff_hook()
        if os.environ.get("GCN_TRACE_DIR"):
            os.makedirs(os.environ["GCN_TRACE_DIR"], exist_ok=True)
            extra["tmpdir"] = os.environ["GCN_TRACE_DIR"]
    LAST_RESULTS = run_bass_kernel_spmd(
        nc, in_maps, list(range(NCORES)), trace=trace, **extra
    )
    out = np.concatenate(
        [np.asarray(LAST_RESULTS.results[c]["outT"]).T for c in range(NCORES)],
        axis=0,
    )
    return np.ascontiguousarray(out.astype(np.float32))
